# revision 34
# baseline (speedup 1.0000x reference)
"""Trainium2 Bass kernel for nn_CausalSelfAttention_72653666779352.

Sharding: 8 cores = 2 batches x 4 kv-groups. Core (b, g) owns
global kv head E=g (q heads 2g, 2g+1) and local kv head F=4+g
(q heads 8+2g, 9+2g). All device compute is in transposed layout
(feature dims on partitions, time on free axis). Matmul operands are
bf16 (fp32r runs ~4x slower per moving column on TRN2 hardware);
PSUM accumulation stays fp32. c_proj is row-parallel: each core
emits a partial [C, T] product; the host sums the 4 partials per
batch (unshard).

Structure (vs the fp32r baseline this replaces):
- software-pipelined per 512-column query chunk: proj(qc+1) is
  emitted before attention(qc) so the tensor engine stays busy while
  rope/rmsnorm elementwise work for chunk qc drains on DVE/Act.
- single scalar-engine activation table (natural_log_exp): rsqrt is
  exp(-0.5*ln(x)), sigmoid is 1/(1+exp(-x)) with the reciprocal on
  DVE. No ACT_TABLE_LOAD switches in steady state.
- k-side rmsnorm is folded into the softmax exp as a per-partition
  scale AP (rsqrt(sum k^2 + 64 eps) = rsqrt(mean+eps)/8 absorbs the
  1/sqrt(hd) score scale too), transposed into per-k-block columns
  with tiny PE transposes.
- causal/window boundary masks are applied inside the score PSUM
  accumulation via an extra -30000-triangle matmul instead of a DVE
  multiply on the exp output.
- y matmul computes both GQA heads of a pair in one instruction
  (3D moving AP) against a [v | ones] stationary so the softmax
  denominator falls out of PSUM rows 64..127.
- c_proj partials are DMA'd straight from PSUM to DRAM.
"""
import contextlib
import numpy as np

B, T, C = 2, 2048, 1024
NH, NKV = 16, 8
HD = 64
VGC = 32
TQC = 512            # tq chunk width
NQC = T // TQC       # 4
NKB = T // 128       # 16
EPS = float(np.finfo(np.float32).eps)
NEG = -30000.0

_CACHE = {}
DEBUG = False


def _ranges(qc, pair, window):
    """kb tiles for (pair, qc): list of (kb, lo, hi, band_lo, mtype).

    Transposed scores tile: partitions tk in [128kb, 128kb+128),
    free cols c -> tq = 512*qc + c.  o = 128*kb - 512*qc.
    causal valid: c >= p + o; window valid (local): c <= p + o + window.
    band offsets are 128-aligned since o and window are.
    """
    out = []
    for kb in range(NKB):
        o = 128 * kb - TQC * qc
        lo = max(0, o)
        hi = TQC if pair == 0 else min(TQC, o + window + 128)
        if lo >= hi:
            continue
        cband = o if 0 <= o < TQC else None
        wband = None
        if pair == 1:
            wb = o + window
            if 0 <= wb < TQC:
                wband = wb
        assert not (cband is not None and wband is not None)
        if cband is not None:
            out.append((kb, lo, hi, cband, "c"))
        elif wband is not None:
            out.append((kb, lo, hi, wband, "w"))
        else:
            out.append((kb, lo, hi, None, None))
    # first tile must cover the full [0, TQC) col range (PSUM has_written)
    first = next(i for i, r in enumerate(out) if r[1] == 0 and r[2] == TQC)
    out[0], out[first] = out[first], out[0]
    return out


def _build(window):
    import concourse.mybir as mybir
    import concourse.tile as tile
    from concourse import bacc

    f32 = mybir.dt.float32
    bf16 = mybir.dt.bfloat16
    nc = bacc.Bacc("TRN2", target_bir_lowering=False, debug=False)

    def din(name, shape, dt=bf16):
        return nc.dram_tensor(name, shape, dt, kind="ExternalInput").ap()

    xT_d = din("xT", [C, T])
    wq_d = din("wq", [C, 256])
    wk_d = din("wk", [C, 128])
    wv_d = din("wv", [C, 128])
    wgate_d = din("wgate", [VGC, 128])
    wproj_d = din("wproj", [256, C])
    m1_d = din("m1", [128, 128])          # block-diag 32x32 ones (q rms sums)
    mk2_d = din("mk2", [128, 2])          # k rms indicator -> [2, t] sums
    ident_d = din("ident", [128, 128])    # bf16 eye (v transpose)
    ident2_d = din("ident2", [2, 2], f32)  # f32 eye (rk transpose)
    maskc_d = din("maskc", [128, 128])    # NEG above causal diag (transposed)
    maskw_d = din("maskw", [128, 128])    # NEG below window diag
    veT_d = din("veT", [128, T])          # 2*ve, per-core heads, transposed
    cos4_d = din("cos4", [128, T])
    sin4_d = din("sin4", [128, T])
    outT_d = nc.dram_tensor("outT", [C, T], bf16, kind="ExternalOutput").ap()
    dbg = {}
    if DEBUG:
        for nm in ("d_q1", "d_q2", "d_kg", "d_kl", "d_yg", "d_yl"):
            dbg[nm] = nc.dram_tensor(nm, [128, T], bf16, kind="ExternalOutput").ap()
        dbg["d_rkT"] = nc.dram_tensor("d_rkT", [128, 32], f32,
                                      kind="ExternalOutput").ap()
        dbg["d_va"] = nc.dram_tensor("d_va", [128, NKB * 256], bf16,
                                     kind="ExternalOutput").ap()

    EXP = mybir.ActivationFunctionType.Exp
    LN = mybir.ActivationFunctionType.Ln
    SQ = mybir.ActivationFunctionType.Square

    # Force a single scalar-engine activation table: keep the real index of
    # natural_log_exp_and_others (it genuinely holds exp/ln/square/copy) and
    # hide those functions from every other table so the table-load pass
    # cannot alternate between per-function tables (1.28us per reload).
    import concourse.bacc as bacc_mod
    from concourse.hw_specs import get_activation_tables as _orig_tables
    _A = mybir.ActivationFunctionType
    _strip = {_A.Exp, _A.Ln, _A.Square, _A.Copy, _A.Identity}

    def _one_table(arch):
        out = {}
        for name, s in _orig_tables(arch).items():
            if name == "natural_log_exp_and_others":
                out[name] = set(s)
            else:
                out[name] = set(s) - _strip
        return out

    bacc_mod.get_activation_tables = _one_table

    with tile.TileContext(nc) as tc, contextlib.ExitStack() as top:
        pers = top.enter_context(tc.tile_pool(name="pers", bufs=1))
        sb = top.enter_context(tc.tile_pool(name="sb", bufs=2))
        ps = top.enter_context(tc.tile_pool(name="ps", bufs=1, space="PSUM"))

        # ---- persistent loads (spread across DGE queues; sync queue kept
        # clear for the first x chunk, which gates the first projections) ----
        wgate_sb = pers.tile([VGC, 128], bf16)
        nc.sync.dma_start(out=wgate_sb, in_=wgate_d)
        wq_sb = pers.tile([128, 8, 256], bf16)
        nc.sync.dma_start(out=wq_sb, in_=wq_d.rearrange("(a p) m -> p a m", p=128))
        wk_sb = pers.tile([128, 8, 128], bf16)
        nc.scalar.dma_start(out=wk_sb, in_=wk_d.rearrange("(a p) m -> p a m", p=128))
        wv_sb = pers.tile([128, 8, 128], bf16)
        nc.scalar.dma_start(out=wv_sb, in_=wv_d.rearrange("(a p) m -> p a m", p=128))
        m1_sb = pers.tile([128, 128], bf16)
        nc.gpsimd.dma_start(out=m1_sb, in_=m1_d)
        mk2_sb = pers.tile([128, 2], bf16)
        nc.gpsimd.dma_start(out=mk2_sb, in_=mk2_d)
        ident_sb = pers.tile([128, 128], bf16)
        nc.gpsimd.dma_start(out=ident_sb, in_=ident_d)
        ident2_sb = pers.tile([2, 2], f32)
        nc.gpsimd.dma_start(out=ident2_sb, in_=ident2_d)
        cos_sb = pers.tile([128, T], bf16)
        nc.scalar.dma_start(out=cos_sb, in_=cos4_d)
        sin_sb = pers.tile([128, T], bf16)
        nc.scalar.dma_start(out=sin_sb, in_=sin4_d)
        maskc_sb = pers.tile([128, 128], bf16)
        nc.gpsimd.dma_start(out=maskc_sb, in_=maskc_d)
        maskw_sb = pers.tile([128, 128], bf16)
        nc.gpsimd.dma_start(out=maskw_sb, in_=maskw_d)
        wp_sb = pers.tile([128, 2, C], bf16)
        nc.gpsimd.dma_start(out=wp_sb, in_=wproj_d.rearrange("(a p) m -> p a m", p=128))

        # persistent activations
        k_g = pers.tile([128, T], bf16)    # [E | E] normed-by-exp-scale k
        k_l = pers.tile([128, T], bf16)    # [F | F]
        yT_g = pers.tile([128, T], bf16)   # [A | B] attention out
        yT_l = pers.tile([128, T], bf16)   # [C | D]
        va = pers.tile([128, NKB, 2, 128], bf16)   # [tpos, kb, pair, v|ones]
        rkT = pers.tile([128, 2, NKB], f32)        # exp scale per (pair, kb)
        nc.vector.memset(va[:, :, :, 64:128], 1.0)
        eps_sb = pers.tile([128, 1], f32)
        nc.vector.memset(eps_sb, EPS)
        eps64_sb = pers.tile([128, 1], f32)
        nc.vector.memset(eps64_sb, EPS * HD)

        xT_r = xT_d.rearrange("(a p) t -> p a t", p=128)

        def load_x(qc):
            ts = slice(qc * TQC, (qc + 1) * TQC)
            xc = sb.tile([128, 8, TQC], bf16, tag="xc", name=f"xc_{qc}")
            if qc == 0:
                # chunked so the first projection can chase the load
                for a in range(8):
                    nc.sync.dma_start(out=xc[:, a, :], in_=xT_r[:, a, ts])
            else:
                nc.sync.dma_start(out=xc, in_=xT_r[:, :, ts])
            vet = sb.tile([128, TQC], bf16, tag="vet", name=f"vet_{qc}")
            nc.sync.dma_start(out=vet, in_=veT_d[:, ts])
            return xc, vet

        def phase1(qc, xc, vet):
            ts = slice(qc * TQC, (qc + 1) * TQC)

            # ---- projections (PSUM ring); gate first so the scalar queue
            # is not head-of-line blocked behind work that needs late groups
            g_ps = ps.tile([128, TQC], f32, tag="pj", bufs=2, name=f"g_{qc}")
            nc.tensor.matmul(g_ps, wgate_sb, xc[0:VGC, 0, :], start=True, stop=True)
            qlo_ps = ps.tile([128, TQC], f32, tag="pj", bufs=2, name=f"qlo_{qc}")
            for a in range(8):
                nc.tensor.matmul(qlo_ps, wq_sb[:, a, 0:128], xc[:, a, :],
                                 start=(a == 0), stop=(a == 7))
            qhi_ps = ps.tile([128, TQC], f32, tag="pj", bufs=2, name=f"qhi_{qc}")
            for a in range(8):
                nc.tensor.matmul(qhi_ps, wq_sb[:, a, 128:256], xc[:, a, :],
                                 start=(a == 0), stop=(a == 7))
            k_ps = ps.tile([128, TQC], f32, tag="pj", bufs=2, name=f"k_{qc}")
            for a in range(8):
                nc.tensor.matmul(k_ps, wk_sb[:, a, :], xc[:, a, :],
                                 start=(a == 0), stop=(a == 7))
            v_ps = ps.tile([128, TQC], f32, tag="pj", bufs=2, name=f"v_{qc}")
            for a in range(8):
                nc.tensor.matmul(v_ps, wv_sb[:, a, :], xc[:, a, :],
                                 start=(a == 0), stop=(a == 7))

            # ---- gate: v_f = v + 2*sigmoid(g) * ve  (veT carries the 2x) ----
            e_g = sb.tile([128, TQC], bf16, tag="eg", name=f"eg_{qc}")
            nc.scalar.activation(e_g, g_ps, EXP, bias=0.0, scale=-1.0)
            den = sb.tile([128, TQC], f32, tag="den", name=f"den_{qc}")
            nc.vector.tensor_scalar_add(den, e_g, 1.0)
            gt = sb.tile([128, TQC], f32, tag="gt", name=f"gt_{qc}")
            nc.vector.reciprocal_approx_fast(gt, den)
            gv = sb.tile([128, TQC], bf16, tag="gv", name=f"gv_{qc}")
            nc.vector.tensor_mul(gv, gt, vet)

            # ---- squares for rms sums (pre-rope; rope preserves norms) ----
            q2a = sb.tile([128, TQC], bf16, tag="q2a", name=f"q2a_{qc}")
            nc.scalar.activation(q2a, qlo_ps, SQ, bias=0.0, scale=1.0)
            q2b = sb.tile([128, TQC], bf16, tag="q2b", name=f"q2b_{qc}")
            nc.scalar.activation(q2b, qhi_ps, SQ, bias=0.0, scale=1.0)
            k2 = sb.tile([128, TQC], bf16, tag="k2", name=f"k2_{qc}")
            nc.scalar.activation(k2, k_ps, SQ, bias=0.0, scale=1.0)

            # ---- q rope (reads PSUM directly) ----
            mc = sb.tile([128, TQC], bf16, tag="mc", name=f"mc_{qc}")
            nc.vector.tensor_mul(mc, qlo_ps, cos_sb[:, ts])
            msn = sb.tile([128, TQC], bf16, tag="msn", name=f"msn_{qc}")
            nc.vector.tensor_mul(msn, qhi_ps, sin_sb[:, ts])
            mc2 = sb.tile([128, TQC], bf16, tag="mc2", name=f"mc2_{qc}")
            nc.vector.tensor_mul(mc2, qhi_ps, cos_sb[:, ts])
            ms2 = sb.tile([128, TQC], bf16, tag="ms2", name=f"ms2_{qc}")
            nc.vector.tensor_mul(ms2, qlo_ps, sin_sb[:, ts])
            rl = sb.tile([128, TQC], bf16, tag="rl", name=f"rl_{qc}")
            rh = sb.tile([128, TQC], bf16, tag="rh", name=f"rh_{qc}")
            for h2 in range(2):
                hs = slice(h2 * 64, h2 * 64 + 64)
                nc.vector.tensor_add(rl[hs, :], mc[hs, :], msn[hs, :])
                nc.vector.tensor_sub(rh[hs, :], mc2[hs, :], ms2[hs, :])

            # ---- k rope (reads PSUM; normalize folded into exp scale) ----
            mck = sb.tile([64, TQC], bf16, tag="mck", name=f"mck_{qc}")
            nc.vector.tensor_mul(mck, k_ps[0:64, :], cos_sb[0:64, ts])
            msk2 = sb.tile([64, TQC], bf16, tag="msk2", name=f"msk2_{qc}")
            nc.vector.tensor_mul(msk2, k_ps[64:128, :], sin_sb[64:128, ts])
            kr = sb.tile([128, TQC], bf16, tag="kr", name=f"kr_{qc}")
            nc.vector.tensor_add(kr[0:64, :], mck, msk2)
            mck2 = sb.tile([64, TQC], bf16, tag="mck2", name=f"mck2_{qc}")
            nc.vector.tensor_mul(mck2, k_ps[64:128, :], cos_sb[64:128, ts])
            msk3 = sb.tile([64, TQC], bf16, tag="msk3", name=f"msk3_{qc}")
            nc.vector.tensor_mul(msk3, k_ps[0:64, :], sin_sb[0:64, ts])
            nc.vector.tensor_sub(kr[64:128, :], mck2, msk3)

            # v_f after the v projection lands
            v_f = sb.tile([128, TQC], bf16, tag="vf", name=f"vf_{qc}")
            nc.vector.tensor_add(v_f, v_ps, gv)

            # permute -> duplicated per-kv-head k tiles
            for half in range(2):
                b0 = half * 64
                eng = nc.sync if half == 0 else nc.scalar
                eng.dma_start(out=k_g[b0:b0 + 32, ts], in_=kr[0:32, :])
                eng.dma_start(out=k_g[b0 + 32:b0 + 64, ts], in_=kr[64:96, :])
                eng.dma_start(out=k_l[b0:b0 + 32, ts], in_=kr[32:64, :])
                eng.dma_start(out=k_l[b0 + 32:b0 + 64, ts], in_=kr[96:128, :])
            return dict(q2a=q2a, q2b=q2b, k2=k2, v_f=v_f, rl=rl, rh=rh)

        def smalls(qc, iv):
            ts = slice(qc * TQC, (qc + 1) * TQC)
            q2a, q2b, k2 = iv["q2a"], iv["q2b"], iv["k2"]
            v_f, rl, rh = iv["v_f"], iv["rl"], iv["rh"]
            ms_ps = ps.tile([128, TQC], f32, tag="pj", bufs=2, name=f"ms_{qc}")
            nc.tensor.matmul(ms_ps, m1_sb, q2a, start=True, stop=False)
            nc.tensor.matmul(ms_ps, m1_sb, q2b, start=False, stop=True)
            lnq = sb.tile([128, TQC], bf16, tag="lnq", name=f"lnq_{qc}")
            nc.scalar.activation(lnq, ms_ps, LN, bias=eps_sb, scale=1.0 / HD)
            rq = sb.tile([128, TQC], bf16, tag="rq", name=f"rq_{qc}")
            nc.scalar.activation(rq, lnq, EXP, bias=0.0, scale=-0.5)

            # normalize directly into per-head q tiles (block muls permute)
            qf1 = sb.tile([128, TQC], bf16, tag="qf1", name=f"qf1_{qc}")
            qf2 = sb.tile([128, TQC], bf16, tag="qf2", name=f"qf2_{qc}")
            for i in range(4):
                dst = qf1 if i < 2 else qf2
                base = (i % 2) * 64
                blk = slice(i * 32, (i + 1) * 32)
                nc.vector.tensor_mul(dst[base:base + 32, :], rl[blk, :], rq[blk, :])
                nc.vector.tensor_mul(dst[base + 32:base + 64, :], rh[blk, :],
                                     rq[blk, :])

            msk_ps = ps.tile([2, TQC], f32, tag="pj", bufs=2, name=f"msk_{qc}")
            nc.tensor.matmul(msk_ps, mk2_sb, k2, start=True, stop=True)
            lnk = sb.tile([2, TQC], f32, tag="lnk", name=f"lnk_{qc}")
            nc.scalar.activation(lnk, msk_ps, LN, bias=eps64_sb[0:2, :], scale=1.0)
            rk2 = sb.tile([2, TQC], f32, tag="rk2", name=f"rk2_{qc}")
            nc.scalar.activation(rk2, lnk, EXP, bias=0.0, scale=-0.5)

            vtr = ps.tile([128, 4, 2, 64], bf16, tag="pj", bufs=2, name=f"vtr_{qc}")
            for j in range(4):
                nc.tensor.transpose(vtr[:, j, :, :],
                                    v_f[:, j * 128:(j + 1) * 128], ident_sb)
            nc.scalar.copy(va[:, qc * 4:(qc + 1) * 4, :, 0:64], vtr)

            ktr = ps.tile([128, 2, 4], f32, tag="pj", bufs=2, name=f"ktr_{qc}")
            for j in range(4):
                nc.tensor.matmul(ktr[:, :, j], rk2[:, j * 128:(j + 1) * 128],
                                 ident2_sb, is_transpose=True)
            nc.scalar.copy(rkT[:, :, qc * 4:(qc + 1) * 4], ktr)

            return qf1, qf2

        def attention(qc, qf1, qf2):
            ts = slice(qc * TQC, (qc + 1) * TQC)
            cfg = [(qf1, k_g, yT_g), (qf2, k_l, yT_l)]
            for pair in range(2):
                qf, kt, yT = cfg[pair]
                rr = _ranges(qc, pair, window)
                yps = ps.tile([128, 2, TQC], f32, tag="y", bufs=1,
                              name=f"y{pair}_{qc}")
                for idx, (kb, lo, hi, band, mt) in enumerate(rr):
                    s2 = ps.tile([128, 2, TQC], f32, tag="s2", bufs=2,
                                 name=f"s{pair}_{qc}_{kb}")
                    ks = slice(kb * 128, (kb + 1) * 128)
                    for h in range(2):
                        hb = h * 64
                        if band is None:
                            nc.tensor.matmul(s2[:, h, lo:hi], kt[hb:hb + 64, ks],
                                             qf[hb:hb + 64, lo:hi],
                                             start=True, stop=True,
                                             tile_position=(hb, 0))
                        else:
                            nc.tensor.matmul(s2[:, h, lo:hi], kt[hb:hb + 64, ks],
                                             qf[hb:hb + 64, lo:hi],
                                             start=True, stop=False,
                                             tile_position=(hb, 0))
                            msk_t = maskc_sb if mt == "c" else maskw_sb
                            nc.tensor.matmul(s2[:, h, band:band + 128], msk_t,
                                             ident_sb, start=False, stop=True)
                    e2 = sb.tile([128, 2, TQC], bf16, tag="e2", bufs=4,
                                 name=f"e{pair}_{qc}_{kb}")
                    nc.scalar.activation(e2[:, :, lo:hi], s2[:, :, lo:hi], EXP,
                                         bias=0.0, scale=rkT[:, pair, kb:kb + 1])
                    for h in range(2):
                        nc.tensor.matmul(yps[:, h, lo:hi], va[:, kb, pair, :],
                                         e2[:, h, lo:hi],
                                         start=(idx == 0),
                                         stop=(idx == len(rr) - 1))
                dent = sb.tile([64, 2, TQC], f32, tag="dent", name=f"den{pair}_{qc}")
                nc.vector.tensor_copy(dent, yps[64:128, :, :])
                rec = sb.tile([64, 2, TQC], f32, tag="rec", name=f"rec{pair}_{qc}")
                nc.vector.reciprocal_approx_fast(rec, dent)
                nc.vector.tensor_mul(yT[0:64, ts], yps[0:64, 0, :], rec[:, 0, :])
                nc.vector.tensor_mul(yT[64:128, ts], yps[0:64, 1, :], rec[:, 1, :])

        def cproj(qc):
            ts = slice(qc * TQC, (qc + 1) * TQC)
            for cb in range(8):
                cs = slice(cb * 128, (cb + 1) * 128)
                pj = ps.tile([128, TQC], f32, tag="pj", bufs=2,
                             name=f"pj_{cb}_{qc}")
                nc.tensor.matmul(pj, wp_sb[:, 0, cs], yT_g[:, ts],
                                 start=True, stop=False)
                nc.tensor.matmul(pj, wp_sb[:, 1, cs], yT_l[:, ts],
                                 start=False, stop=True)
                ot = sb.tile([128, TQC], bf16, tag="ot", bufs=3,
                             name=f"ot_{cb}_{qc}")
                if cb % 2 == 0:
                    nc.vector.tensor_copy(ot, pj)
                else:
                    nc.scalar.copy(ot, pj)
                eng = nc.sync if cb % 2 == 0 else nc.gpsimd
                eng.dma_start(out=outT_d[cs, ts], in_=ot)

        qfs = {}
        ivs = {}
        xcs = {0: load_x(0)}
        for qc in range(NQC):
            if qc + 1 < NQC:
                xcs[qc + 1] = load_x(qc + 1)
            ivs[qc] = phase1(qc, *xcs.pop(qc))
            if qc >= 1:
                attention(qc - 1, *qfs[qc - 1])
                cproj(qc - 1)
            qfs[qc] = smalls(qc, ivs.pop(qc))
        attention(NQC - 1, *qfs[NQC - 1])
        cproj(NQC - 1)

        if DEBUG:
            for nm, t in [("d_kg", k_g), ("d_kl", k_l),
                          ("d_yg", yT_g), ("d_yl", yT_l)]:
                nc.sync.dma_start(out=dbg[nm], in_=t)
            nc.sync.dma_start(out=dbg["d_q1"][:, 0:TQC], in_=qfs[NQC - 1][0])
            nc.sync.dma_start(out=dbg["d_q2"][:, 0:TQC], in_=qfs[NQC - 1][1])
            nc.sync.dma_start(out=dbg["d_rkT"], in_=rkT)
            nc.sync.dma_start(out=dbg["d_va"],
                              in_=va.rearrange("p a b c -> p (a b c)"))

    nc.compile()
    return nc


def _host_inputs(x, ve, cos, sin, Wq, Wk, Wv, Wproj, Wgate):
    """Per-core input maps (core = b*4 + g)."""
    import ml_dtypes
    bf16 = ml_dtypes.bfloat16

    x = np.asarray(x, np.float32)
    ve = np.asarray(ve, np.float32)
    cos = np.asarray(cos, np.float32).reshape(T, -1)   # [T, 32]
    sin = np.asarray(sin, np.float32).reshape(T, -1)
    Wq = np.asarray(Wq, np.float32)
    Wk = np.asarray(Wk, np.float32)
    Wv = np.asarray(Wv, np.float32)
    Wproj = np.asarray(Wproj, np.float32)
    Wgate = np.asarray(Wgate, np.float32)

    cos4 = np.ascontiguousarray(np.tile(cos.T, (4, 1))).astype(bf16)  # [128, T]
    sin4 = np.ascontiguousarray(np.tile(sin.T, (4, 1))).astype(bf16)
    ident = np.eye(128, dtype=np.float32).astype(bf16)
    ident2 = np.eye(2, dtype=np.float32)
    m1 = np.kron(np.eye(4, dtype=np.float32),
                 np.ones((32, 32), np.float32)).astype(bf16)
    mk2 = np.zeros((128, 2), np.float32)
    for p in range(128):
        mk2[p, (p % 64) // 32] = 1.0
    mk2 = mk2.astype(bf16)
    aa = np.arange(128)[:, None]
    bb = np.arange(128)[None, :]
    # mask add M[p,c'] = lhsT[c',p]: causal masked when c' < p
    maskc = (NEG * (aa < bb)).astype(np.float32).astype(bf16)
    maskw = (NEG * (aa > bb)).astype(np.float32).astype(bf16)

    xT = [np.ascontiguousarray(x[b].T).astype(bf16) for b in range(B)]
    maps = []
    for core in range(8):
        b, g = divmod(core, 4)
        heads = [2 * g, 2 * g + 1, 8 + 2 * g, 9 + 2 * g]  # A B C D
        E, F = g, 4 + g
        qcols = [64 * h + d for h in heads for d in range(32)] + \
                [64 * h + 32 + d for h in heads for d in range(32)]
        kcols = [64 * h + d for h in (E, F) for d in range(32)] + \
                [64 * h + 32 + d for h in (E, F) for d in range(32)]
        vcols = [64 * E + d for d in range(64)] + [64 * F + d for d in range(64)]
        prow = [64 * h + d for h in heads for d in range(64)]
        wgate_b = np.ascontiguousarray(
            np.concatenate([np.repeat(Wgate[:, E:E + 1], 64, 1),
                            np.repeat(Wgate[:, F:F + 1], 64, 1)], 1)).astype(bf16)
        veT = np.ascontiguousarray((2.0 * ve[b][:, vcols]).T).astype(bf16)
        maps.append({
            "xT": xT[b],
            "wq": np.ascontiguousarray(Wq[:, qcols]).astype(bf16),
            "wk": np.ascontiguousarray(Wk[:, kcols]).astype(bf16),
            "wv": np.ascontiguousarray(Wv[:, vcols]).astype(bf16),
            "wgate": wgate_b,
            "wproj": np.ascontiguousarray(Wproj[prow, :]).astype(bf16),
            "m1": m1, "mk2": mk2, "ident": ident, "ident2": ident2,
            "maskc": maskc, "maskw": maskw,
            "veT": veT, "cos4": cos4, "sin4": sin4,
        })
    return maps


def _run(inputs, trace=False):
    from concourse.bass_utils import run_bass_kernel_spmd
    window = int(inputs["window"])
    assert window == 512, f"kernel tuned for window=512, got {window}"
    if window not in _CACHE:
        _CACHE[window] = _build(window)
    nc = _CACHE[window]
    maps = _host_inputs(inputs["x"], inputs["ve"], inputs["cos"], inputs["sin"],
                        inputs["Wq"], inputs["Wk"], inputs["Wv"],
                        inputs["Wproj"], inputs["Wgate"])
    res = run_bass_kernel_spmd(nc, maps, list(range(8)), trace=trace)
    y = np.zeros((B, T, C), dtype=np.float32)
    for core, r in enumerate(res.results):
        b = core // 4
        y[b] += np.asarray(r["outT"]).astype(np.float32).T
    return y, res


def kernel(**inputs):
    y, _ = _run(inputs, trace=False)
    return y


# revision 37
# speedup vs baseline: 1.1193x; 1.1193x over previous
"""Trainium2 Bass kernel for nn_CausalSelfAttention_72653666779352.

Sharding: 8 cores = 2 batches x 4 kv-groups. Core (b, g) owns
global kv head E=g (q heads 2g, 2g+1) and local kv head F=4+g
(q heads 8+2g, 9+2g). All device compute is in transposed layout
(feature dims on partitions, time on free axis). Matmul operands are
bf16 (fp32r runs ~4x slower per moving column on TRN2 hardware);
PSUM accumulation stays fp32. c_proj is row-parallel: each core
emits a partial [C, T] product; the host sums the 4 partials per
batch (unshard).

Structure (vs the fp32r baseline this replaces):
- software-pipelined per 512-column query chunk: proj(qc+1) is
  emitted before attention(qc) so the tensor engine stays busy while
  rope/rmsnorm elementwise work for chunk qc drains on DVE/Act.
- single scalar-engine activation table (natural_log_exp): rsqrt is
  exp(-0.5*ln(x)), sigmoid is 1/(1+exp(-x)) with the reciprocal on
  DVE. No ACT_TABLE_LOAD switches in steady state.
- k-side rmsnorm is folded into the softmax exp as a per-partition
  scale AP (rsqrt(sum k^2 + 64 eps) = rsqrt(mean+eps)/8 absorbs the
  1/sqrt(hd) score scale too), transposed into per-k-block columns
  with tiny PE transposes.
- causal/window boundary masks are applied inside the score PSUM
  accumulation via an extra -30000-triangle matmul instead of a DVE
  multiply on the exp output.
- y matmul computes both GQA heads of a pair in one instruction
  (3D moving AP) against a [v | ones] stationary so the softmax
  denominator falls out of PSUM rows 64..127.
- c_proj partials are DMA'd straight from PSUM to DRAM.
"""
import contextlib
import numpy as np

B, T, C = 2, 2048, 1024
NH, NKV = 16, 8
HD = 64
VGC = 32
TQC = 512            # tq chunk width
NQC = T // TQC       # 4
NKB = T // 128       # 16
EPS = float(np.finfo(np.float32).eps)
NEG = -30000.0

_CACHE = {}
DEBUG = False


def _ranges(qc, pair, window):
    """kb tiles for (pair, qc): list of (kb, lo, hi, band_lo, mtype).

    Transposed scores tile: partitions tk in [128kb, 128kb+128),
    free cols c -> tq = 512*qc + c.  o = 128*kb - 512*qc.
    causal valid: c >= p + o; window valid (local): c <= p + o + window.
    band offsets are 128-aligned since o and window are.
    """
    out = []
    for kb in range(NKB):
        o = 128 * kb - TQC * qc
        lo = max(0, o)
        hi = TQC if pair == 0 else min(TQC, o + window + 128)
        if lo >= hi:
            continue
        cband = o if 0 <= o < TQC else None
        wband = None
        if pair == 1:
            wb = o + window
            if 0 <= wb < TQC:
                wband = wb
        assert not (cband is not None and wband is not None)
        if cband is not None:
            out.append((kb, lo, hi, cband, "c"))
        elif wband is not None:
            out.append((kb, lo, hi, wband, "w"))
        else:
            out.append((kb, lo, hi, None, None))
    # first tile must cover the full [0, TQC) col range (PSUM has_written)
    first = next(i for i, r in enumerate(out) if r[1] == 0 and r[2] == TQC)
    out[0], out[first] = out[first], out[0]
    return out


def _build(window):
    import concourse.mybir as mybir
    import concourse.tile as tile
    from concourse import bacc

    f32 = mybir.dt.float32
    bf16 = mybir.dt.bfloat16
    nc = bacc.Bacc("TRN2", target_bir_lowering=False, debug=False)

    def din(name, shape, dt=bf16):
        return nc.dram_tensor(name, shape, dt, kind="ExternalInput").ap()

    xT_d = din("xT", [C, T])
    wq_d = din("wq", [C, 256])
    wk_d = din("wk", [C, 128])
    wv_d = din("wv", [C, 128])
    wgate_d = din("wgate", [VGC, 128])
    wproj_d = din("wproj", [256, C])
    m1_d = din("m1", [128, 128])          # block-diag 32x32 ones (q rms sums)
    mk2_d = din("mk2", [128, 2])          # k rms indicator -> [2, t] sums
    ident_d = din("ident", [128, 128])    # bf16 eye (v transpose)
    ident2_d = din("ident2", [2, 2], f32)  # f32 eye (rk transpose)
    maskc_d = din("maskc", [128, 128])    # NEG above causal diag (transposed)
    maskw_d = din("maskw", [128, 128])    # NEG below window diag
    veT_d = din("veT", [128, T])          # 2*ve, per-core heads, transposed
    cos4_d = din("cos4", [128, T])
    sin4_d = din("sin4", [128, T])
    outT_d = nc.dram_tensor("outT", [C, T], bf16, kind="ExternalOutput").ap()
    dbg = {}
    if DEBUG:
        for nm in ("d_q1", "d_q2", "d_kg", "d_kl", "d_yg", "d_yl"):
            dbg[nm] = nc.dram_tensor(nm, [128, T], bf16, kind="ExternalOutput").ap()
        dbg["d_rkT"] = nc.dram_tensor("d_rkT", [128, 32], f32,
                                      kind="ExternalOutput").ap()
        dbg["d_va"] = nc.dram_tensor("d_va", [128, NKB * 256], bf16,
                                     kind="ExternalOutput").ap()

    EXP = mybir.ActivationFunctionType.Exp
    LN = mybir.ActivationFunctionType.Ln
    SQ = mybir.ActivationFunctionType.Square

    # Force a single scalar-engine activation table: keep the real index of
    # natural_log_exp_and_others (it genuinely holds exp/ln/square/copy) and
    # hide those functions from every other table so the table-load pass
    # cannot alternate between per-function tables (1.28us per reload).
    import concourse.bacc as bacc_mod
    from concourse.hw_specs import get_activation_tables as _orig_tables
    _A = mybir.ActivationFunctionType
    _strip = {_A.Exp, _A.Ln, _A.Square, _A.Copy, _A.Identity}

    def _one_table(arch):
        out = {}
        for name, s in _orig_tables(arch).items():
            if name == "natural_log_exp_and_others":
                out[name] = set(s)
            else:
                out[name] = set(s) - _strip
        return out

    bacc_mod.get_activation_tables = _one_table

    with tile.TileContext(nc) as tc, contextlib.ExitStack() as top:
        pers = top.enter_context(tc.tile_pool(name="pers", bufs=1))
        sb = top.enter_context(tc.tile_pool(name="sb", bufs=2))
        ps = top.enter_context(tc.tile_pool(name="ps", bufs=1, space="PSUM"))

        # ---- persistent loads (spread across DGE queues; sync queue kept
        # clear for the first x chunk, which gates the first projections) ----
        wgate_sb = pers.tile([VGC, 128], bf16)
        nc.sync.dma_start(out=wgate_sb, in_=wgate_d)
        wq_sb = pers.tile([128, 8, 256], bf16)
        nc.sync.dma_start(out=wq_sb, in_=wq_d.rearrange("(a p) m -> p a m", p=128))
        wk_sb = pers.tile([128, 8, 128], bf16)
        nc.scalar.dma_start(out=wk_sb, in_=wk_d.rearrange("(a p) m -> p a m", p=128))
        wv_sb = pers.tile([128, 8, 128], bf16)
        nc.scalar.dma_start(out=wv_sb, in_=wv_d.rearrange("(a p) m -> p a m", p=128))
        m1_sb = pers.tile([128, 128], bf16)
        nc.gpsimd.dma_start(out=m1_sb, in_=m1_d)
        mk2_sb = pers.tile([128, 2], bf16)
        nc.gpsimd.dma_start(out=mk2_sb, in_=mk2_d)
        ident_sb = pers.tile([128, 128], bf16)
        nc.gpsimd.dma_start(out=ident_sb, in_=ident_d)
        ident2_sb = pers.tile([2, 2], f32)
        nc.gpsimd.dma_start(out=ident2_sb, in_=ident2_d)
        cos_sb = pers.tile([128, T], bf16)
        nc.scalar.dma_start(out=cos_sb, in_=cos4_d)
        sin_sb = pers.tile([128, T], bf16)
        nc.scalar.dma_start(out=sin_sb, in_=sin4_d)
        maskc_sb = pers.tile([128, 128], bf16)
        nc.gpsimd.dma_start(out=maskc_sb, in_=maskc_d)
        maskw_sb = pers.tile([128, 128], bf16)
        nc.gpsimd.dma_start(out=maskw_sb, in_=maskw_d)
        wp_sb = pers.tile([128, 2, C], bf16)
        nc.gpsimd.dma_start(out=wp_sb, in_=wproj_d.rearrange("(a p) m -> p a m", p=128))

        # persistent activations
        k_g = pers.tile([128, T], bf16)    # [E | E] normed-by-exp-scale k
        k_l = pers.tile([128, T], bf16)    # [F | F]
        yT_g = pers.tile([128, T], bf16)   # [A | B] attention out
        yT_l = pers.tile([128, T], bf16)   # [C | D]
        va = pers.tile([128, NKB, 2, 128], bf16)   # [tpos, kb, pair, v|ones]
        rkT = pers.tile([128, 2, NKB], f32)        # exp scale per (pair, kb)
        nc.vector.memset(va[:, :, :, 64:128], 1.0)
        eps_sb = pers.tile([128, 1], f32)
        nc.vector.memset(eps_sb, EPS)
        eps64_sb = pers.tile([128, 1], f32)
        nc.vector.memset(eps64_sb, EPS * HD)

        xT_r = xT_d.rearrange("(a p) t -> p a t", p=128)

        def load_x(qc):
            ts = slice(qc * TQC, (qc + 1) * TQC)
            xc = sb.tile([128, 8, TQC], bf16, tag="xc", name=f"xc_{qc}")
            if qc == 0:
                # chunked so the first projection can chase the load
                for a in range(8):
                    nc.sync.dma_start(out=xc[:, a, :], in_=xT_r[:, a, ts])
            else:
                nc.sync.dma_start(out=xc, in_=xT_r[:, :, ts])
            vet = sb.tile([128, TQC], bf16, tag="vet", name=f"vet_{qc}")
            nc.sync.dma_start(out=vet, in_=veT_d[:, ts])
            return xc, vet

        def phase1(qc, xc, vet):
            ts = slice(qc * TQC, (qc + 1) * TQC)

            # ---- projections (PSUM ring); gate first so the scalar queue
            # is not head-of-line blocked behind work that needs late groups
            g_ps = ps.tile([128, TQC], f32, tag="pj", bufs=2, name=f"g_{qc}")
            nc.tensor.matmul(g_ps, wgate_sb, xc[0:VGC, 0, :], start=True, stop=True)
            qlo_ps = ps.tile([128, TQC], f32, tag="pj", bufs=2, name=f"qlo_{qc}")
            for a in range(8):
                nc.tensor.matmul(qlo_ps, wq_sb[:, a, 0:128], xc[:, a, :],
                                 start=(a == 0), stop=(a == 7))
            qhi_ps = ps.tile([128, TQC], f32, tag="pj", bufs=2, name=f"qhi_{qc}")
            for a in range(8):
                nc.tensor.matmul(qhi_ps, wq_sb[:, a, 128:256], xc[:, a, :],
                                 start=(a == 0), stop=(a == 7))
            k_ps = ps.tile([128, TQC], f32, tag="pj", bufs=2, name=f"k_{qc}")
            for a in range(8):
                nc.tensor.matmul(k_ps, wk_sb[:, a, :], xc[:, a, :],
                                 start=(a == 0), stop=(a == 7))
            v_ps = ps.tile([128, TQC], f32, tag="pj", bufs=2, name=f"v_{qc}")
            for a in range(8):
                nc.tensor.matmul(v_ps, wv_sb[:, a, :], xc[:, a, :],
                                 start=(a == 0), stop=(a == 7))

            # ---- gate: v_f = v + 2*sigmoid(g) * ve  (veT carries the 2x) ----
            e_g = sb.tile([128, TQC], bf16, tag="eg", name=f"eg_{qc}")
            nc.scalar.activation(e_g, g_ps, EXP, bias=0.0, scale=-1.0)
            den = sb.tile([128, TQC], f32, tag="den", name=f"den_{qc}")
            nc.vector.tensor_scalar_add(den, e_g, 1.0)
            gt = sb.tile([128, TQC], f32, tag="gt", name=f"gt_{qc}")
            nc.vector.reciprocal_approx_fast(gt, den)
            gv = sb.tile([128, TQC], bf16, tag="gv", name=f"gv_{qc}")
            nc.vector.tensor_mul(gv, gt, vet)

            # ---- squares for rms sums (pre-rope; rope preserves norms) ----
            q2a = sb.tile([128, TQC], bf16, tag="q2a", name=f"q2a_{qc}")
            nc.scalar.activation(q2a, qlo_ps, SQ, bias=0.0, scale=1.0)
            q2b = sb.tile([128, TQC], bf16, tag="q2b", name=f"q2b_{qc}")
            nc.scalar.activation(q2b, qhi_ps, SQ, bias=0.0, scale=1.0)
            k2 = sb.tile([128, TQC], bf16, tag="k2", name=f"k2_{qc}")
            nc.scalar.activation(k2, k_ps, SQ, bias=0.0, scale=1.0)

            # ---- q rope (reads PSUM directly) ----
            mc = sb.tile([128, TQC], bf16, tag="mc", name=f"mc_{qc}")
            nc.vector.tensor_mul(mc, qlo_ps, cos_sb[:, ts])
            msn = sb.tile([128, TQC], bf16, tag="msn", name=f"msn_{qc}")
            nc.vector.tensor_mul(msn, qhi_ps, sin_sb[:, ts])
            mc2 = sb.tile([128, TQC], bf16, tag="mc2", name=f"mc2_{qc}")
            nc.vector.tensor_mul(mc2, qhi_ps, cos_sb[:, ts])
            ms2 = sb.tile([128, TQC], bf16, tag="ms2", name=f"ms2_{qc}")
            nc.vector.tensor_mul(ms2, qlo_ps, sin_sb[:, ts])
            rl = sb.tile([128, TQC], bf16, tag="rl", name=f"rl_{qc}")
            rh = sb.tile([128, TQC], bf16, tag="rh", name=f"rh_{qc}")
            for h2 in range(2):
                hs = slice(h2 * 64, h2 * 64 + 64)
                nc.vector.tensor_add(rl[hs, :], mc[hs, :], msn[hs, :])
                nc.vector.tensor_sub(rh[hs, :], mc2[hs, :], ms2[hs, :])

            # ---- k rope (reads PSUM; normalize folded into exp scale) ----
            mck = sb.tile([64, TQC], bf16, tag="mck", name=f"mck_{qc}")
            nc.vector.tensor_mul(mck, k_ps[0:64, :], cos_sb[0:64, ts])
            msk2 = sb.tile([64, TQC], bf16, tag="msk2", name=f"msk2_{qc}")
            nc.vector.tensor_mul(msk2, k_ps[64:128, :], sin_sb[64:128, ts])
            kr = sb.tile([128, TQC], bf16, tag="kr", name=f"kr_{qc}")
            nc.vector.tensor_add(kr[0:64, :], mck, msk2)
            mck2 = sb.tile([64, TQC], bf16, tag="mck2", name=f"mck2_{qc}")
            nc.vector.tensor_mul(mck2, k_ps[64:128, :], cos_sb[64:128, ts])
            msk3 = sb.tile([64, TQC], bf16, tag="msk3", name=f"msk3_{qc}")
            nc.vector.tensor_mul(msk3, k_ps[0:64, :], sin_sb[0:64, ts])
            nc.vector.tensor_sub(kr[64:128, :], mck2, msk3)

            # v_f after the v projection lands
            v_f = sb.tile([128, TQC], bf16, tag="vf", name=f"vf_{qc}")
            nc.vector.tensor_add(v_f, v_ps, gv)

            # permute -> duplicated per-kv-head k tiles
            for half in range(2):
                b0 = half * 64
                eng = nc.sync if half == 0 else nc.scalar
                eng.dma_start(out=k_g[b0:b0 + 32, ts], in_=kr[0:32, :])
                eng.dma_start(out=k_g[b0 + 32:b0 + 64, ts], in_=kr[64:96, :])
                eng.dma_start(out=k_l[b0:b0 + 32, ts], in_=kr[32:64, :])
                eng.dma_start(out=k_l[b0 + 32:b0 + 64, ts], in_=kr[96:128, :])

            ms_ps = ps.tile([128, TQC], f32, tag="pj", bufs=2, name=f"ms_{qc}")
            nc.tensor.matmul(ms_ps, m1_sb, q2a, start=True, stop=False)
            nc.tensor.matmul(ms_ps, m1_sb, q2b, start=False, stop=True)
            lnq = sb.tile([128, TQC], bf16, tag="lnq", name=f"lnq_{qc}")
            nc.scalar.activation(lnq, ms_ps, LN, bias=eps_sb, scale=1.0 / HD)
            rq = sb.tile([128, TQC], bf16, tag="rq", name=f"rq_{qc}")
            nc.scalar.activation(rq, lnq, EXP, bias=0.0, scale=-0.5)

            # normalize directly into per-head q tiles (block muls permute)
            qf1 = sb.tile([128, TQC], bf16, tag="qf1", name=f"qf1_{qc}")
            qf2 = sb.tile([128, TQC], bf16, tag="qf2", name=f"qf2_{qc}")
            for i in range(4):
                dst = qf1 if i < 2 else qf2
                base = (i % 2) * 64
                blk = slice(i * 32, (i + 1) * 32)
                nc.vector.tensor_mul(dst[base:base + 32, :], rl[blk, :], rq[blk, :])
                nc.vector.tensor_mul(dst[base + 32:base + 64, :], rh[blk, :],
                                     rq[blk, :])

            msk_ps = ps.tile([2, TQC], f32, tag="pj", bufs=2, name=f"msk_{qc}")
            nc.tensor.matmul(msk_ps, mk2_sb, k2, start=True, stop=True)
            lnk = sb.tile([2, TQC], f32, tag="lnk", name=f"lnk_{qc}")
            nc.scalar.activation(lnk, msk_ps, LN, bias=eps64_sb[0:2, :], scale=1.0)
            rk2 = sb.tile([2, TQC], f32, tag="rk2", name=f"rk2_{qc}")
            nc.scalar.activation(rk2, lnk, EXP, bias=0.0, scale=-0.5)

            vtr = ps.tile([128, 4, 2, 64], bf16, tag="pj", bufs=2, name=f"vtr_{qc}")
            for j in range(4):
                nc.tensor.transpose(vtr[:, j, :, :],
                                    v_f[:, j * 128:(j + 1) * 128], ident_sb)
            nc.scalar.copy(va[:, qc * 4:(qc + 1) * 4, :, 0:64], vtr)

            return qf1, qf2, rk2

        def ktr_late(qc, rk2):
            ktr = ps.tile([128, 2, 4], f32, tag="pj", bufs=2, name=f"ktr_{qc}")
            for j in range(4):
                nc.tensor.matmul(ktr[:, :, j], rk2[:, j * 128:(j + 1) * 128],
                                 ident2_sb, is_transpose=True)
            nc.scalar.copy(rkT[:, :, qc * 4:(qc + 1) * 4], ktr)

        def attention(qc, qf1, qf2):
            ts = slice(qc * TQC, (qc + 1) * TQC)
            cfg = [(qf1, k_g, yT_g), (qf2, k_l, yT_l)]
            for pair in range(2):
                qf, kt, yT = cfg[pair]
                rr = _ranges(qc, pair, window)
                yps = ps.tile([128, 2, TQC], f32, tag="y", bufs=1,
                              name=f"y{pair}_{qc}")
                for idx, (kb, lo, hi, band, mt) in enumerate(rr):
                    s2 = ps.tile([128, 2, TQC], f32, tag="s2", bufs=2,
                                 name=f"s{pair}_{qc}_{kb}")
                    ks = slice(kb * 128, (kb + 1) * 128)
                    for h in range(2):
                        hb = h * 64
                        if band is None:
                            nc.tensor.matmul(s2[:, h, lo:hi], kt[hb:hb + 64, ks],
                                             qf[hb:hb + 64, lo:hi],
                                             start=True, stop=True,
                                             tile_position=(hb, 0))
                        else:
                            nc.tensor.matmul(s2[:, h, lo:hi], kt[hb:hb + 64, ks],
                                             qf[hb:hb + 64, lo:hi],
                                             start=True, stop=False,
                                             tile_position=(hb, 0))
                            msk_t = maskc_sb if mt == "c" else maskw_sb
                            nc.tensor.matmul(s2[:, h, band:band + 128], msk_t,
                                             ident_sb, start=False, stop=True)
                    e2 = sb.tile([128, 2, TQC], bf16, tag="e2", bufs=4,
                                 name=f"e{pair}_{qc}_{kb}")
                    nc.scalar.activation(e2[:, :, lo:hi], s2[:, :, lo:hi], EXP,
                                         bias=0.0, scale=rkT[:, pair, kb:kb + 1])
                    for h in range(2):
                        nc.tensor.matmul(yps[:, h, lo:hi], va[:, kb, pair, :],
                                         e2[:, h, lo:hi],
                                         start=(idx == 0),
                                         stop=(idx == len(rr) - 1))
                dent = sb.tile([64, 2, TQC], f32, tag="dent", name=f"den{pair}_{qc}")
                nc.vector.tensor_copy(dent, yps[64:128, :, :])
                rec = sb.tile([64, 2, TQC], f32, tag="rec", name=f"rec{pair}_{qc}")
                nc.vector.reciprocal_approx_fast(rec, dent)
                nc.vector.tensor_mul(yT[0:64, ts], yps[0:64, 0, :], rec[:, 0, :])
                nc.vector.tensor_mul(yT[64:128, ts], yps[0:64, 1, :], rec[:, 1, :])

        def cproj(qc):
            ts = slice(qc * TQC, (qc + 1) * TQC)
            for cb in range(8):
                cs = slice(cb * 128, (cb + 1) * 128)
                pj = ps.tile([128, TQC], f32, tag="pj", bufs=2,
                             name=f"pj_{cb}_{qc}")
                nc.tensor.matmul(pj, wp_sb[:, 0, cs], yT_g[:, ts],
                                 start=True, stop=False)
                nc.tensor.matmul(pj, wp_sb[:, 1, cs], yT_l[:, ts],
                                 start=False, stop=True)
                ot = sb.tile([128, TQC], bf16, tag="ot", bufs=3,
                             name=f"ot_{cb}_{qc}")
                if cb % 2 == 0:
                    nc.vector.tensor_copy(ot, pj)
                else:
                    nc.scalar.copy(ot, pj)
                eng = nc.sync if cb % 2 == 0 else nc.gpsimd
                eng.dma_start(out=outT_d[cs, ts], in_=ot)

        qfs = {}
        xcs = {0: load_x(0)}
        for qc in range(NQC):
            if qc + 1 < NQC:
                xcs[qc + 1] = load_x(qc + 1)
            r = phase1(qc, *xcs.pop(qc))
            qfs[qc] = r[0], r[1]
            if qc >= 1:
                attention(qc - 1, *qfs[qc - 1])
                cproj(qc - 1)
            ktr_late(qc, r[2])
        attention(NQC - 1, *qfs[NQC - 1])
        cproj(NQC - 1)

        if DEBUG:
            for nm, t in [("d_kg", k_g), ("d_kl", k_l),
                          ("d_yg", yT_g), ("d_yl", yT_l)]:
                nc.sync.dma_start(out=dbg[nm], in_=t)
            nc.sync.dma_start(out=dbg["d_q1"][:, 0:TQC], in_=qfs[NQC - 1][0])
            nc.sync.dma_start(out=dbg["d_q2"][:, 0:TQC], in_=qfs[NQC - 1][1])
            nc.sync.dma_start(out=dbg["d_rkT"], in_=rkT)
            nc.sync.dma_start(out=dbg["d_va"],
                              in_=va.rearrange("p a b c -> p (a b c)"))

    nc.compile()
    return nc


def _host_inputs(x, ve, cos, sin, Wq, Wk, Wv, Wproj, Wgate):
    """Per-core input maps (core = b*4 + g)."""
    import ml_dtypes
    bf16 = ml_dtypes.bfloat16

    x = np.asarray(x, np.float32)
    ve = np.asarray(ve, np.float32)
    cos = np.asarray(cos, np.float32).reshape(T, -1)   # [T, 32]
    sin = np.asarray(sin, np.float32).reshape(T, -1)
    Wq = np.asarray(Wq, np.float32)
    Wk = np.asarray(Wk, np.float32)
    Wv = np.asarray(Wv, np.float32)
    Wproj = np.asarray(Wproj, np.float32)
    Wgate = np.asarray(Wgate, np.float32)

    cos4 = np.ascontiguousarray(np.tile(cos.T, (4, 1))).astype(bf16)  # [128, T]
    sin4 = np.ascontiguousarray(np.tile(sin.T, (4, 1))).astype(bf16)
    ident = np.eye(128, dtype=np.float32).astype(bf16)
    ident2 = np.eye(2, dtype=np.float32)
    m1 = np.kron(np.eye(4, dtype=np.float32),
                 np.ones((32, 32), np.float32)).astype(bf16)
    mk2 = np.zeros((128, 2), np.float32)
    for p in range(128):
        mk2[p, (p % 64) // 32] = 1.0
    mk2 = mk2.astype(bf16)
    aa = np.arange(128)[:, None]
    bb = np.arange(128)[None, :]
    # mask add M[p,c'] = lhsT[c',p]: causal masked when c' < p
    maskc = (NEG * (aa < bb)).astype(np.float32).astype(bf16)
    maskw = (NEG * (aa > bb)).astype(np.float32).astype(bf16)

    xT = [np.ascontiguousarray(x[b].T).astype(bf16) for b in range(B)]
    maps = []
    for core in range(8):
        b, g = divmod(core, 4)
        heads = [2 * g, 2 * g + 1, 8 + 2 * g, 9 + 2 * g]  # A B C D
        E, F = g, 4 + g
        qcols = [64 * h + d for h in heads for d in range(32)] + \
                [64 * h + 32 + d for h in heads for d in range(32)]
        kcols = [64 * h + d for h in (E, F) for d in range(32)] + \
                [64 * h + 32 + d for h in (E, F) for d in range(32)]
        vcols = [64 * E + d for d in range(64)] + [64 * F + d for d in range(64)]
        prow = [64 * h + d for h in heads for d in range(64)]
        wgate_b = np.ascontiguousarray(
            np.concatenate([np.repeat(Wgate[:, E:E + 1], 64, 1),
                            np.repeat(Wgate[:, F:F + 1], 64, 1)], 1)).astype(bf16)
        veT = np.ascontiguousarray((2.0 * ve[b][:, vcols]).T).astype(bf16)
        maps.append({
            "xT": xT[b],
            "wq": np.ascontiguousarray(Wq[:, qcols]).astype(bf16),
            "wk": np.ascontiguousarray(Wk[:, kcols]).astype(bf16),
            "wv": np.ascontiguousarray(Wv[:, vcols]).astype(bf16),
            "wgate": wgate_b,
            "wproj": np.ascontiguousarray(Wproj[prow, :]).astype(bf16),
            "m1": m1, "mk2": mk2, "ident": ident, "ident2": ident2,
            "maskc": maskc, "maskw": maskw,
            "veT": veT, "cos4": cos4, "sin4": sin4,
        })
    return maps


def _run(inputs, trace=False):
    from concourse.bass_utils import run_bass_kernel_spmd
    window = int(inputs["window"])
    assert window == 512, f"kernel tuned for window=512, got {window}"
    if window not in _CACHE:
        _CACHE[window] = _build(window)
    nc = _CACHE[window]
    maps = _host_inputs(inputs["x"], inputs["ve"], inputs["cos"], inputs["sin"],
                        inputs["Wq"], inputs["Wk"], inputs["Wv"],
                        inputs["Wproj"], inputs["Wgate"])
    res = run_bass_kernel_spmd(nc, maps, list(range(8)), trace=trace)
    y = np.zeros((B, T, C), dtype=np.float32)
    for core, r in enumerate(res.results):
        b = core // 4
        y[b] += np.asarray(r["outT"]).astype(np.float32).T
    return y, res


def kernel(**inputs):
    y, _ = _run(inputs, trace=False)
    return y


# revision 38
# speedup vs baseline: 1.1732x; 1.0481x over previous
"""Trainium2 Bass kernel for nn_CausalSelfAttention_72653666779352.

Sharding: 8 cores = 2 batches x 4 kv-groups. Core (b, g) owns
global kv head E=g (q heads 2g, 2g+1) and local kv head F=4+g
(q heads 8+2g, 9+2g). All device compute is in transposed layout
(feature dims on partitions, time on free axis). Matmul operands are
bf16 (fp32r runs ~4x slower per moving column on TRN2 hardware);
PSUM accumulation stays fp32. c_proj is row-parallel: each core
emits a partial [C, T] product; the host sums the 4 partials per
batch (unshard).

Structure (vs the fp32r baseline this replaces):
- software-pipelined per 512-column query chunk: proj(qc+1) is
  emitted before attention(qc) so the tensor engine stays busy while
  rope/rmsnorm elementwise work for chunk qc drains on DVE/Act.
- single scalar-engine activation table (natural_log_exp): rsqrt is
  exp(-0.5*ln(x)), sigmoid is 1/(1+exp(-x)) with the reciprocal on
  DVE. No ACT_TABLE_LOAD switches in steady state.
- k-side rmsnorm is folded into the softmax exp as a per-partition
  scale AP (rsqrt(sum k^2 + 64 eps) = rsqrt(mean+eps)/8 absorbs the
  1/sqrt(hd) score scale too), transposed into per-k-block columns
  with tiny PE transposes.
- causal/window boundary masks are applied inside the score PSUM
  accumulation via an extra -30000-triangle matmul instead of a DVE
  multiply on the exp output.
- y matmul computes both GQA heads of a pair in one instruction
  (3D moving AP) against a [v | ones] stationary so the softmax
  denominator falls out of PSUM rows 64..127.
- c_proj partials are DMA'd straight from PSUM to DRAM.
"""
import contextlib
import numpy as np

B, T, C = 2, 2048, 1024
NH, NKV = 16, 8
HD = 64
VGC = 32
TQC = 512            # tq chunk width
NQC = T // TQC       # 4
NKB = T // 128       # 16
EPS = float(np.finfo(np.float32).eps)
NEG = -30000.0

_CACHE = {}
DEBUG = False


def _ranges(qc, pair, window):
    """kb tiles for (pair, qc): list of (kb, lo, hi, band_lo, mtype).

    Transposed scores tile: partitions tk in [128kb, 128kb+128),
    free cols c -> tq = 512*qc + c.  o = 128*kb - 512*qc.
    causal valid: c >= p + o; window valid (local): c <= p + o + window.
    band offsets are 128-aligned since o and window are.
    """
    out = []
    for kb in range(NKB):
        o = 128 * kb - TQC * qc
        lo = max(0, o)
        hi = TQC if pair == 0 else min(TQC, o + window + 128)
        if lo >= hi:
            continue
        cband = o if 0 <= o < TQC else None
        wband = None
        if pair == 1:
            wb = o + window
            if 0 <= wb < TQC:
                wband = wb
        assert not (cband is not None and wband is not None)
        if cband is not None:
            out.append((kb, lo, hi, cband, "c"))
        elif wband is not None:
            out.append((kb, lo, hi, wband, "w"))
        else:
            out.append((kb, lo, hi, None, None))
    # first tile must cover the full [0, TQC) col range (PSUM has_written)
    first = next(i for i, r in enumerate(out) if r[1] == 0 and r[2] == TQC)
    out[0], out[first] = out[first], out[0]
    return out


def _build(window):
    import concourse.mybir as mybir
    import concourse.tile as tile
    from concourse import bacc

    f32 = mybir.dt.float32
    bf16 = mybir.dt.bfloat16
    nc = bacc.Bacc("TRN2", target_bir_lowering=False, debug=False)

    def din(name, shape, dt=bf16):
        return nc.dram_tensor(name, shape, dt, kind="ExternalInput").ap()

    xT_d = din("xT", [C, T])
    wq_d = din("wq", [C, 256])
    wk_d = din("wk", [C, 128])
    wv_d = din("wv", [C, 128])
    wgate_d = din("wgate", [VGC, 128])
    wproj_d = din("wproj", [256, C])
    m1_d = din("m1", [128, 128])          # block-diag 32x32 ones (q rms sums)
    mk2_d = din("mk2", [128, 2])          # k rms indicator -> [2, t] sums
    ident_d = din("ident", [128, 128])    # bf16 eye (v transpose)
    ident2_d = din("ident2", [2, 2], f32)  # f32 eye (rk transpose)
    maskc_d = din("maskc", [128, 128])    # NEG above causal diag (transposed)
    maskw_d = din("maskw", [128, 128])    # NEG below window diag
    veT_d = din("veT", [128, T])          # 2*ve, per-core heads, transposed
    cos4_d = din("cos4", [128, T])
    sin4_d = din("sin4", [128, T])
    outT_d = nc.dram_tensor("outT", [C, T], bf16, kind="ExternalOutput").ap()
    dbg = {}
    if DEBUG:
        for nm in ("d_q1", "d_q2", "d_kg", "d_kl", "d_yg", "d_yl"):
            dbg[nm] = nc.dram_tensor(nm, [128, T], bf16, kind="ExternalOutput").ap()
        dbg["d_rkT"] = nc.dram_tensor("d_rkT", [128, 32], f32,
                                      kind="ExternalOutput").ap()
        dbg["d_va"] = nc.dram_tensor("d_va", [128, NKB * 256], bf16,
                                     kind="ExternalOutput").ap()

    EXP = mybir.ActivationFunctionType.Exp
    LN = mybir.ActivationFunctionType.Ln
    SQ = mybir.ActivationFunctionType.Square

    # Force a single scalar-engine activation table: keep the real index of
    # natural_log_exp_and_others (it genuinely holds exp/ln/square/copy) and
    # hide those functions from every other table so the table-load pass
    # cannot alternate between per-function tables (1.28us per reload).
    import concourse.bacc as bacc_mod
    from concourse.hw_specs import get_activation_tables as _orig_tables
    _A = mybir.ActivationFunctionType
    _strip = {_A.Exp, _A.Ln, _A.Square, _A.Copy, _A.Identity}

    def _one_table(arch):
        out = {}
        for name, s in _orig_tables(arch).items():
            if name == "natural_log_exp_and_others":
                out[name] = set(s)
            else:
                out[name] = set(s) - _strip
        return out

    bacc_mod.get_activation_tables = _one_table

    with tile.TileContext(nc) as tc, contextlib.ExitStack() as top:
        pers = top.enter_context(tc.tile_pool(name="pers", bufs=1))
        sb = top.enter_context(tc.tile_pool(name="sb", bufs=2))
        ps = top.enter_context(tc.tile_pool(name="ps", bufs=1, space="PSUM"))

        # ---- persistent loads (spread across DGE queues; sync queue kept
        # clear for the first x chunk, which gates the first projections) ----
        wgate_sb = pers.tile([VGC, 128], bf16)
        nc.sync.dma_start(out=wgate_sb, in_=wgate_d)
        wq_sb = pers.tile([128, 8, 256], bf16)
        nc.sync.dma_start(out=wq_sb, in_=wq_d.rearrange("(a p) m -> p a m", p=128))
        wk_sb = pers.tile([128, 8, 128], bf16)
        nc.scalar.dma_start(out=wk_sb, in_=wk_d.rearrange("(a p) m -> p a m", p=128))
        wv_sb = pers.tile([128, 8, 128], bf16)
        nc.scalar.dma_start(out=wv_sb, in_=wv_d.rearrange("(a p) m -> p a m", p=128))
        m1_sb = pers.tile([128, 128], bf16)
        nc.gpsimd.dma_start(out=m1_sb, in_=m1_d)
        mk2_sb = pers.tile([128, 2], bf16)
        nc.gpsimd.dma_start(out=mk2_sb, in_=mk2_d)
        ident_sb = pers.tile([128, 128], bf16)
        nc.gpsimd.dma_start(out=ident_sb, in_=ident_d)
        ident2_sb = pers.tile([2, 2], f32)
        nc.gpsimd.dma_start(out=ident2_sb, in_=ident2_d)
        cos_sb = pers.tile([128, T], bf16)
        nc.scalar.dma_start(out=cos_sb, in_=cos4_d)
        sin_sb = pers.tile([128, T], bf16)
        nc.scalar.dma_start(out=sin_sb, in_=sin4_d)
        maskc_sb = pers.tile([128, 128], bf16)
        nc.gpsimd.dma_start(out=maskc_sb, in_=maskc_d)
        maskw_sb = pers.tile([128, 128], bf16)
        nc.gpsimd.dma_start(out=maskw_sb, in_=maskw_d)
        wp_sb = pers.tile([128, 2, C], bf16)
        nc.gpsimd.dma_start(out=wp_sb, in_=wproj_d.rearrange("(a p) m -> p a m", p=128))

        # persistent activations
        k_g = pers.tile([128, T], bf16)    # [E | E] normed-by-exp-scale k
        k_l = pers.tile([128, T], bf16)    # [F | F]
        yT_g = pers.tile([128, T], bf16)   # [A | B] attention out
        yT_l = pers.tile([128, T], bf16)   # [C | D]
        va = pers.tile([128, NKB, 2, 128], bf16)   # [tpos, kb, pair, v|ones]
        rkT = pers.tile([128, 2, NKB], f32)        # exp scale per (pair, kb)
        nc.vector.memset(va[:, :, :, 64:128], 1.0)
        eps_sb = pers.tile([128, 1], f32)
        nc.vector.memset(eps_sb, EPS)
        eps64_sb = pers.tile([128, 1], f32)
        nc.vector.memset(eps64_sb, EPS * HD)

        xT_r = xT_d.rearrange("(a p) t -> p a t", p=128)

        def load_x(qc):
            ts = slice(qc * TQC, (qc + 1) * TQC)
            xc = sb.tile([128, 8, TQC], bf16, tag="xc", name=f"xc_{qc}")
            if qc == 0:
                # chunked so the first projection can chase the load
                for a in range(8):
                    nc.sync.dma_start(out=xc[:, a, :], in_=xT_r[:, a, ts])
            else:
                nc.sync.dma_start(out=xc, in_=xT_r[:, :, ts])
            vet = sb.tile([128, TQC], bf16, tag="vet", name=f"vet_{qc}")
            nc.sync.dma_start(out=vet, in_=veT_d[:, ts])
            return xc, vet

        def phase1(qc, xc, vet):
            ts = slice(qc * TQC, (qc + 1) * TQC)

            # ---- projections (PSUM ring); gate first so the scalar queue
            # is not head-of-line blocked behind work that needs late groups
            g_ps = ps.tile([128, TQC], f32, tag="pj", bufs=2, name=f"g_{qc}")
            nc.tensor.matmul(g_ps, wgate_sb, xc[0:VGC, 0, :], start=True, stop=True)
            qlo_ps = ps.tile([128, TQC], f32, tag="pj", bufs=2, name=f"qlo_{qc}")
            for a in range(8):
                nc.tensor.matmul(qlo_ps, wq_sb[:, a, 0:128], xc[:, a, :],
                                 start=(a == 0), stop=(a == 7))
            qhi_ps = ps.tile([128, TQC], f32, tag="pj", bufs=2, name=f"qhi_{qc}")
            for a in range(8):
                nc.tensor.matmul(qhi_ps, wq_sb[:, a, 128:256], xc[:, a, :],
                                 start=(a == 0), stop=(a == 7))
            k_ps = ps.tile([128, TQC], f32, tag="pj", bufs=2, name=f"k_{qc}")
            for a in range(8):
                nc.tensor.matmul(k_ps, wk_sb[:, a, :], xc[:, a, :],
                                 start=(a == 0), stop=(a == 7))
            v_ps = ps.tile([128, TQC], f32, tag="pj", bufs=2, name=f"v_{qc}")
            for a in range(8):
                nc.tensor.matmul(v_ps, wv_sb[:, a, :], xc[:, a, :],
                                 start=(a == 0), stop=(a == 7))

            # ---- gate: v_f = v + 2*sigmoid(g) * ve  (veT carries the 2x) ----
            e_g = sb.tile([128, TQC], bf16, tag="eg", name=f"eg_{qc}")
            nc.scalar.activation(e_g, g_ps, EXP, bias=0.0, scale=-1.0)
            den = sb.tile([128, TQC], f32, tag="den", name=f"den_{qc}")
            nc.vector.tensor_scalar_add(den, e_g, 1.0)
            gt = sb.tile([128, TQC], f32, tag="gt", name=f"gt_{qc}")
            nc.vector.reciprocal_approx_fast(gt, den)
            gv = sb.tile([128, TQC], bf16, tag="gv", name=f"gv_{qc}")
            nc.vector.tensor_mul(gv, gt, vet)

            # ---- squares for rms sums (pre-rope; rope preserves norms) ----
            q2a = sb.tile([128, TQC], bf16, tag="q2a", name=f"q2a_{qc}")
            nc.scalar.activation(q2a, qlo_ps, SQ, bias=0.0, scale=1.0)
            q2b = sb.tile([128, TQC], bf16, tag="q2b", name=f"q2b_{qc}")
            nc.scalar.activation(q2b, qhi_ps, SQ, bias=0.0, scale=1.0)
            k2 = sb.tile([128, TQC], bf16, tag="k2", name=f"k2_{qc}")
            nc.scalar.activation(k2, k_ps, SQ, bias=0.0, scale=1.0)

            # ---- q rope (reads PSUM directly) ----
            mc = sb.tile([128, TQC], bf16, tag="mc", name=f"mc_{qc}")
            nc.vector.tensor_mul(mc, qlo_ps, cos_sb[:, ts])
            msn = sb.tile([128, TQC], bf16, tag="msn", name=f"msn_{qc}")
            nc.vector.tensor_mul(msn, qhi_ps, sin_sb[:, ts])
            mc2 = sb.tile([128, TQC], bf16, tag="mc2", name=f"mc2_{qc}")
            nc.vector.tensor_mul(mc2, qhi_ps, cos_sb[:, ts])
            ms2 = sb.tile([128, TQC], bf16, tag="ms2", name=f"ms2_{qc}")
            nc.vector.tensor_mul(ms2, qlo_ps, sin_sb[:, ts])
            rl = sb.tile([128, TQC], bf16, tag="rl", name=f"rl_{qc}")
            rh = sb.tile([128, TQC], bf16, tag="rh", name=f"rh_{qc}")
            for h2 in range(2):
                hs = slice(h2 * 64, h2 * 64 + 64)
                nc.vector.tensor_add(rl[hs, :], mc[hs, :], msn[hs, :])
                nc.vector.tensor_sub(rh[hs, :], mc2[hs, :], ms2[hs, :])

            # ---- k rope (reads PSUM; normalize folded into exp scale) ----
            mck = sb.tile([64, TQC], bf16, tag="mck", name=f"mck_{qc}")
            nc.vector.tensor_mul(mck, k_ps[0:64, :], cos_sb[0:64, ts])
            msk2 = sb.tile([64, TQC], bf16, tag="msk2", name=f"msk2_{qc}")
            nc.vector.tensor_mul(msk2, k_ps[64:128, :], sin_sb[64:128, ts])
            kr = sb.tile([128, TQC], bf16, tag="kr", name=f"kr_{qc}")
            nc.vector.tensor_add(kr[0:64, :], mck, msk2)
            mck2 = sb.tile([64, TQC], bf16, tag="mck2", name=f"mck2_{qc}")
            nc.vector.tensor_mul(mck2, k_ps[64:128, :], cos_sb[64:128, ts])
            msk3 = sb.tile([64, TQC], bf16, tag="msk3", name=f"msk3_{qc}")
            nc.vector.tensor_mul(msk3, k_ps[0:64, :], sin_sb[0:64, ts])
            nc.vector.tensor_sub(kr[64:128, :], mck2, msk3)

            # v_f after the v projection lands
            v_f = sb.tile([128, TQC], bf16, tag="vf", name=f"vf_{qc}")
            nc.vector.tensor_add(v_f, v_ps, gv)

            # permute -> duplicated per-kv-head k tiles
            for half in range(2):
                b0 = half * 64
                eng = nc.sync if half == 0 else nc.scalar
                eng.dma_start(out=k_g[b0:b0 + 32, ts], in_=kr[0:32, :])
                eng.dma_start(out=k_g[b0 + 32:b0 + 64, ts], in_=kr[64:96, :])
                eng.dma_start(out=k_l[b0:b0 + 32, ts], in_=kr[32:64, :])
                eng.dma_start(out=k_l[b0 + 32:b0 + 64, ts], in_=kr[96:128, :])

            ms_ps = ps.tile([128, TQC], f32, tag="pj", bufs=2, name=f"ms_{qc}")
            nc.tensor.matmul(ms_ps, m1_sb, q2a, start=True, stop=False)
            nc.tensor.matmul(ms_ps, m1_sb, q2b, start=False, stop=True)
            lnq = sb.tile([128, TQC], bf16, tag="lnq", name=f"lnq_{qc}")
            nc.scalar.activation(lnq, ms_ps, LN, bias=eps_sb, scale=1.0 / HD)
            rq = sb.tile([128, TQC], bf16, tag="rq", name=f"rq_{qc}")
            nc.scalar.activation(rq, lnq, EXP, bias=0.0, scale=-0.5)

            # normalize directly into per-head q tiles (block muls permute)
            qf1 = sb.tile([128, TQC], bf16, tag="qf1", name=f"qf1_{qc}")
            qf2 = sb.tile([128, TQC], bf16, tag="qf2", name=f"qf2_{qc}")
            for i in range(4):
                dst = qf1 if i < 2 else qf2
                base = (i % 2) * 64
                blk = slice(i * 32, (i + 1) * 32)
                nc.vector.tensor_mul(dst[base:base + 32, :], rl[blk, :], rq[blk, :])
                nc.vector.tensor_mul(dst[base + 32:base + 64, :], rh[blk, :],
                                     rq[blk, :])

            msk_ps = ps.tile([2, TQC], f32, tag="pj", bufs=2, name=f"msk_{qc}")
            nc.tensor.matmul(msk_ps, mk2_sb, k2, start=True, stop=True)
            lnk = sb.tile([2, TQC], f32, tag="lnk", name=f"lnk_{qc}")
            nc.scalar.activation(lnk, msk_ps, LN, bias=eps64_sb[0:2, :], scale=1.0)
            rk2 = sb.tile([2, TQC], f32, tag="rk2", name=f"rk2_{qc}")
            nc.scalar.activation(rk2, lnk, EXP, bias=0.0, scale=-0.5)

            vtr = ps.tile([128, 4, 2, 64], bf16, tag="pj", bufs=2, name=f"vtr_{qc}")
            for j in range(4):
                nc.tensor.transpose(vtr[:, j, :, :],
                                    v_f[:, j * 128:(j + 1) * 128], ident_sb)
            nc.scalar.copy(va[:, qc * 4:(qc + 1) * 4, :, 0:64], vtr)

            return qf1, qf2, rk2

        def ktr_late(qc, rk2):
            ktr = ps.tile([128, 2, 4], f32, tag="pj", bufs=2, name=f"ktr_{qc}")
            for j in range(4):
                nc.tensor.matmul(ktr[:, :, j], rk2[:, j * 128:(j + 1) * 128],
                                 ident2_sb, is_transpose=True)
            nc.scalar.copy(rkT[:, :, qc * 4:(qc + 1) * 4], ktr)

        def attention(qc, qf1, qf2):
            ts = slice(qc * TQC, (qc + 1) * TQC)
            cfg = [(qf1, k_g, yT_g), (qf2, k_l, yT_l)]
            for pair in range(2):
                qf, kt, yT = cfg[pair]
                rr = _ranges(qc, pair, window)
                yps = ps.tile([128, 2, TQC], f32, tag="y", bufs=1,
                              name=f"y{pair}_{qc}")
                for idx, (kb, lo, hi, band, mt) in enumerate(rr):
                    s2 = ps.tile([128, 2, TQC], f32, tag="s2", bufs=2,
                                 name=f"s{pair}_{qc}_{kb}")
                    ks = slice(kb * 128, (kb + 1) * 128)
                    for h in range(2):
                        hb = h * 64
                        if band is None:
                            nc.tensor.matmul(s2[:, h, lo:hi], kt[hb:hb + 64, ks],
                                             qf[hb:hb + 64, lo:hi],
                                             start=True, stop=True,
                                             tile_position=(hb, 0))
                        else:
                            nc.tensor.matmul(s2[:, h, lo:hi], kt[hb:hb + 64, ks],
                                             qf[hb:hb + 64, lo:hi],
                                             start=True, stop=False,
                                             tile_position=(hb, 0))
                            msk_t = maskc_sb if mt == "c" else maskw_sb
                            nc.tensor.matmul(s2[:, h, band:band + 128], msk_t,
                                             ident_sb, start=False, stop=True)
                    e2 = sb.tile([128, 2, TQC], bf16, tag="e2", bufs=4,
                                 name=f"e{pair}_{qc}_{kb}")
                    nc.scalar.activation(e2[:, :, lo:hi], s2[:, :, lo:hi], EXP,
                                         bias=0.0, scale=rkT[:, pair, kb:kb + 1])
                    for h in range(2):
                        nc.tensor.matmul(yps[:, h, lo:hi], va[:, kb, pair, :],
                                         e2[:, h, lo:hi],
                                         start=(idx == 0),
                                         stop=(idx == len(rr) - 1))
                dent = sb.tile([64, 2, TQC], f32, tag="dent", name=f"den{pair}_{qc}")
                nc.vector.tensor_copy(dent, yps[64:128, :, :])
                rec = sb.tile([64, 2, TQC], f32, tag="rec", name=f"rec{pair}_{qc}")
                nc.vector.reciprocal_approx_fast(rec, dent)
                nc.vector.tensor_mul(yT[0:64, ts], yps[0:64, 0, :], rec[:, 0, :])
                nc.vector.tensor_mul(yT[64:128, ts], yps[0:64, 1, :], rec[:, 1, :])

        def cproj(qc):
            ts = slice(qc * TQC, (qc + 1) * TQC)
            for cb in range(8):
                cs = slice(cb * 128, (cb + 1) * 128)
                pj = ps.tile([128, TQC], f32, tag="pj", bufs=2,
                             name=f"pj_{cb}_{qc}")
                nc.tensor.matmul(pj, wp_sb[:, 0, cs], yT_g[:, ts],
                                 start=True, stop=False)
                nc.tensor.matmul(pj, wp_sb[:, 1, cs], yT_l[:, ts],
                                 start=False, stop=True)
                ot = sb.tile([128, TQC], bf16, tag="ot", bufs=3,
                             name=f"ot_{cb}_{qc}")
                if cb % 2 == 0:
                    nc.vector.tensor_copy(ot, pj)
                else:
                    nc.scalar.copy(ot, pj)
                eng = nc.sync if cb % 2 == 0 else nc.gpsimd
                eng.dma_start(out=outT_d[cs, ts], in_=ot)

        qfs = {}
        xcs = {0: load_x(0)}
        for qc in range(NQC):
            if qc + 1 < NQC:
                xcs[qc + 1] = load_x(qc + 1)
            r = phase1(qc, *xcs.pop(qc))
            qfs[qc] = r[0], r[1]
            ktr_late(qc, r[2])
            if qc >= 1:
                attention(qc - 1, *qfs[qc - 1])
                cproj(qc - 1)
        attention(NQC - 1, *qfs[NQC - 1])
        cproj(NQC - 1)

        if DEBUG:
            for nm, t in [("d_kg", k_g), ("d_kl", k_l),
                          ("d_yg", yT_g), ("d_yl", yT_l)]:
                nc.sync.dma_start(out=dbg[nm], in_=t)
            nc.sync.dma_start(out=dbg["d_q1"][:, 0:TQC], in_=qfs[NQC - 1][0])
            nc.sync.dma_start(out=dbg["d_q2"][:, 0:TQC], in_=qfs[NQC - 1][1])
            nc.sync.dma_start(out=dbg["d_rkT"], in_=rkT)
            nc.sync.dma_start(out=dbg["d_va"],
                              in_=va.rearrange("p a b c -> p (a b c)"))

    nc.compile()
    return nc


def _host_inputs(x, ve, cos, sin, Wq, Wk, Wv, Wproj, Wgate):
    """Per-core input maps (core = b*4 + g)."""
    import ml_dtypes
    bf16 = ml_dtypes.bfloat16

    x = np.asarray(x, np.float32)
    ve = np.asarray(ve, np.float32)
    cos = np.asarray(cos, np.float32).reshape(T, -1)   # [T, 32]
    sin = np.asarray(sin, np.float32).reshape(T, -1)
    Wq = np.asarray(Wq, np.float32)
    Wk = np.asarray(Wk, np.float32)
    Wv = np.asarray(Wv, np.float32)
    Wproj = np.asarray(Wproj, np.float32)
    Wgate = np.asarray(Wgate, np.float32)

    cos4 = np.ascontiguousarray(np.tile(cos.T, (4, 1))).astype(bf16)  # [128, T]
    sin4 = np.ascontiguousarray(np.tile(sin.T, (4, 1))).astype(bf16)
    ident = np.eye(128, dtype=np.float32).astype(bf16)
    ident2 = np.eye(2, dtype=np.float32)
    m1 = np.kron(np.eye(4, dtype=np.float32),
                 np.ones((32, 32), np.float32)).astype(bf16)
    mk2 = np.zeros((128, 2), np.float32)
    for p in range(128):
        mk2[p, (p % 64) // 32] = 1.0
    mk2 = mk2.astype(bf16)
    aa = np.arange(128)[:, None]
    bb = np.arange(128)[None, :]
    # mask add M[p,c'] = lhsT[c',p]: causal masked when c' < p
    maskc = (NEG * (aa < bb)).astype(np.float32).astype(bf16)
    maskw = (NEG * (aa > bb)).astype(np.float32).astype(bf16)

    xT = [np.ascontiguousarray(x[b].T).astype(bf16) for b in range(B)]
    maps = []
    for core in range(8):
        b, g = divmod(core, 4)
        heads = [2 * g, 2 * g + 1, 8 + 2 * g, 9 + 2 * g]  # A B C D
        E, F = g, 4 + g
        qcols = [64 * h + d for h in heads for d in range(32)] + \
                [64 * h + 32 + d for h in heads for d in range(32)]
        kcols = [64 * h + d for h in (E, F) for d in range(32)] + \
                [64 * h + 32 + d for h in (E, F) for d in range(32)]
        vcols = [64 * E + d for d in range(64)] + [64 * F + d for d in range(64)]
        prow = [64 * h + d for h in heads for d in range(64)]
        wgate_b = np.ascontiguousarray(
            np.concatenate([np.repeat(Wgate[:, E:E + 1], 64, 1),
                            np.repeat(Wgate[:, F:F + 1], 64, 1)], 1)).astype(bf16)
        veT = np.ascontiguousarray((2.0 * ve[b][:, vcols]).T).astype(bf16)
        maps.append({
            "xT": xT[b],
            "wq": np.ascontiguousarray(Wq[:, qcols]).astype(bf16),
            "wk": np.ascontiguousarray(Wk[:, kcols]).astype(bf16),
            "wv": np.ascontiguousarray(Wv[:, vcols]).astype(bf16),
            "wgate": wgate_b,
            "wproj": np.ascontiguousarray(Wproj[prow, :]).astype(bf16),
            "m1": m1, "mk2": mk2, "ident": ident, "ident2": ident2,
            "maskc": maskc, "maskw": maskw,
            "veT": veT, "cos4": cos4, "sin4": sin4,
        })
    return maps


def _run(inputs, trace=False):
    from concourse.bass_utils import run_bass_kernel_spmd
    window = int(inputs["window"])
    assert window == 512, f"kernel tuned for window=512, got {window}"
    if window not in _CACHE:
        _CACHE[window] = _build(window)
    nc = _CACHE[window]
    maps = _host_inputs(inputs["x"], inputs["ve"], inputs["cos"], inputs["sin"],
                        inputs["Wq"], inputs["Wk"], inputs["Wv"],
                        inputs["Wproj"], inputs["Wgate"])
    res = run_bass_kernel_spmd(nc, maps, list(range(8)), trace=trace)
    y = np.zeros((B, T, C), dtype=np.float32)
    for core, r in enumerate(res.results):
        b = core // 4
        y[b] += np.asarray(r["outT"]).astype(np.float32).T
    return y, res


def kernel(**inputs):
    y, _ = _run(inputs, trace=False)
    return y
